# revision 13
# baseline (speedup 1.0000x reference)
"""Multi-head causal linear attention (B=1, N=2048, D=1024, H=16) on 8 trn2 cores.

Math: reference computes, per head (e=64):
    q = softmax(q_raw, -1) * e**-0.5 ;  k = exp(k_raw)
    out_n = (q_n . KV_n) / (q_n . (kcum_n + EPS)),  KV_n = sum_{j<=n} k_j v_j^T
Because both numerator and denominator are linear in q_n, the softmax
normalization and the e**-0.5 scale cancel exactly; only u = exp(q_raw)
matters.  The EPS term contributes <1e-6 relative and is dropped.  The
v-bias is folded into the value vectors (v' = v + b_v): since the
attention weights sum to 1, out(v') = out(v) + b_v exactly.

Per-core work (head-parallel, 2 heads/core):
  * k-outer q/k projection: per 128-wide d-slice, two matmuls (W
    stationary, x^T moving) gated on that slice's [W | x] DMA, so the
    input stream paces the first token tile instead of stalling the PE
    behind a monolithic transfer.  v projection runs k-inner right
    after, reusing the q PSUM bank.
  * Chunked causal linear attention (chunk=128): intra (masked QK^T V)
    + inter (running KV state) recurrence.  The KV state is kept
    block-diagonal over the 2 heads (off-blocks zeroed once) so inter
    is a single matmul over both heads and the per-chunk state update
    is one vector add.
"""

import os
from contextlib import ExitStack

import numpy as np

import concourse.bass as bass
import concourse.mybir as mybir
import concourse.tile as tile
from concourse import bacc
from concourse._compat import with_exitstack
from concourse.bass import ts

FP32 = mybir.dt.float32
BF16 = mybir.dt.bfloat16

B, N, D, H = 1, 2048, 1024, 16
E = D // H          # 64 head dim
NCORES = 8
HPC = H // NCORES   # 2 heads per core
KT = D // 128       # 8 contraction (d) slices
TT = 512            # token tile (projection granularity)
NTT = N // TT       # 4
C = 128             # chunk (tokens) for the causal recurrence
CPT = TT // C       # 4 chunks per token tile
NC = N // C         # 16 chunks total
WB = 3 * 128        # W columns per k-slice (q|k|v blocks)
KSL = WB + TT       # mega cols per k-slice: [Wq|Wk|Wv|X0]
NJUNK = 8           # PE warm-up matmuls (bridge DMA arm latency ~3.4us)

Exp = mybir.ActivationFunctionType.Exp
Copy = mybir.ActivationFunctionType.Copy
Ident = mybir.ActivationFunctionType.Identity


@with_exitstack
def _emit(ctx: ExitStack, tc, io):
    nc = tc.nc
    mega_d, cf_d, msk_d, xt1_d, xt23_d, out_d = io

    const = ctx.enter_context(tc.tile_pool(name="const", bufs=1))
    chain = ctx.enter_context(tc.tile_pool(name="chain", bufs=2))
    smtp = ctx.enter_context(tc.tile_pool(name="smtp", bufs=4))
    small = ctx.enter_context(tc.tile_pool(name="small", bufs=3))
    outp = ctx.enter_context(tc.tile_pool(name="outp", bufs=3))
    pproj = ctx.enter_context(tc.tile_pool(name="pproj", bufs=1, space="PSUM"))
    ps_s = ctx.enter_context(tc.tile_pool(name="ps_s", bufs=2, space="PSUM"))
    ps_tr = ctx.enter_context(tc.tile_pool(name="ps_tr", bufs=1, space="PSUM"))
    ps_out = ctx.enter_context(tc.tile_pool(name="ps_out", bufs=2, space="PSUM"))
    ps_dp = ctx.enter_context(tc.tile_pool(name="ps_dp", bufs=1, space="PSUM"))

    # ---- persistent SBUF ----
    # mega: [ ident 128 | (Wq|Wk|Wv 384 | xt0 512) x 8 ]
    mega_sb = const.tile([128, 128 + KT * KSL], BF16)
    cf_sb = const.tile([128, 3], FP32)             # [bq|bk|bv]
    msk_sb = const.tile([128, C], BF16)            # causal mask [j, i], 1 iff j<=i
    xtr_sb = const.tile([128, (NTT - 1) * KT * TT], BF16)  # xt tt=1..3, (tt k t)

    id_sb = mega_sb[:, 0:128]
    bq_sb = cf_sb[:, 0:1]
    bk_sb = cf_sb[:, 1:2]
    bv_sb = cf_sb[:, 2:3]

    def w_ap(k, f):
        base = 128 + k * KSL + f * 128
        return mega_sb[:, base : base + 128]

    def xt_ap(tt, k):
        if tt == 0:
            base = 128 + k * KSL + WB
            return mega_sb[:, base : base + TT]
        base = (tt - 1) * KT * TT + k * TT
        return xtr_sb[:, base : base + TT]

    # block-diagonal KV state: delta PSUM ping-pong + fp32 master
    # ping-pong, all with off-diagonal head blocks zeroed once.
    dpp_bank = ps_dp.tile([128, 512], FP32)
    dpp = dpp_bank[:, 0 : 2 * HPC * (E + 1)].rearrange(
        "p (a g e) -> p a g e", a=2, g=HPC
    )
    nc.vector.memset(dpp[:, :, :, :], 0.0)
    kvm = const.tile([128, 2, HPC, E + 1], FP32)
    nc.vector.memset(kvm[:, :, :, :], 0.0)

    # PE warm-up bridge: junk matmuls on zeros keep the HAM activity window
    # busy while inputs stream in, so real matmuls start at 2.4 GHz.
    scratch = const.tile([128, TT], BF16)
    nc.gpsimd.memset(scratch[:, :], 0.0)
    junk_ps = ps_s.tile([128, TT], FP32, tag="s", name="junk")
    for j in range(NJUNK):
        nc.tensor.matmul(
            junk_ps[:, :],
            lhsT=scratch[:, 0:128],
            rhs=scratch[:, :],
            start=True,
            stop=True,
        )

    # input DMAs, strictly in consumption order on one engine so the HBM
    # stream is never stolen by data needed later. k-slice-pair
    # granularity gates the tile-0 projection.
    M0 = 128  # ident rides with the first pair
    for kp in range(KT // 2):
        lo = (M0 if kp else 0) + kp * 2 * KSL
        hi = M0 + (kp + 1) * 2 * KSL
        nc.sync.dma_start(mega_sb[:, lo:hi], mega_d[:, lo:hi])
        if kp == 1:
            nc.sync.dma_start(cf_sb[:, :], cf_d[:, :])
            nc.sync.dma_start(msk_sb[:, :], msk_d[:, :])
    T3 = 3 * TT
    nc.sync.dma_start(xtr_sb[:, 0:T3], xt1_d[:, 0:T3])
    nc.sync.dma_start(xtr_sb[:, T3 : 2 * T3], xt1_d[:, T3 : 2 * T3])
    nc.sync.dma_start(xtr_sb[:, 2 * T3 : KT * TT], xt1_d[:, 2 * T3 : KT * TT])
    nc.sync.dma_start(
        xtr_sb[:, KT * TT : 2 * KT * TT], xt23_d[:, 0 : KT * TT]
    )
    nc.sync.dma_start(
        xtr_sb[:, 2 * KT * TT : 3 * KT * TT], xt23_d[:, KT * TT : 2 * KT * TT]
    )

    kv_prev = None   # bf16 block-diagonal [128, HPC, E+1] (matmul operand)
    dma_flip = [0]

    st = [dict() for _ in range(NTT)]

    def emit_qk_slice(tt, k):
        s = st[tt]
        if k == 0:
            s["qps"] = pproj.tile([128, TT], FP32, tag="q", name=f"qps{tt}")
            s["kps"] = pproj.tile([128, TT], FP32, tag="k", name=f"kps{tt}")
        first, last = k == 0, k == KT - 1
        nc.tensor.matmul(
            s["kps"][:, :], lhsT=w_ap(k, 1), rhs=xt_ap(tt, k),
            start=first, stop=last,
        )
        nc.tensor.matmul(
            s["qps"][:, :], lhsT=w_ap(k, 0), rhs=xt_ap(tt, k),
            start=first, stop=last,
        )

    def emit_act(tt):
        s = st[tt]
        s["EkT"] = EkT = chain.tile([128, TT], BF16, tag="EkT", name=f"EkT{tt}")
        nc.scalar.activation(EkT[:, :], s["kps"][:, :], Exp, bias=bk_sb[:, 0:1])
        s["UT"] = UT = chain.tile([128, TT], BF16, tag="UT", name=f"UT{tt}")
        nc.scalar.activation(UT[:, :], s["qps"][:, :], Exp, bias=bq_sb[:, 0:1])

    def emit_v(tt):
        # v projection k-inner, reusing the q PSUM bank (freed by UT act)
        s = st[tt]
        vps = pproj.tile([128, TT], FP32, tag="q", name=f"vps{tt}")
        for k in range(KT):
            nc.tensor.matmul(
                vps[:, :], lhsT=w_ap(k, 2), rhs=xt_ap(tt, k),
                start=(k == 0), stop=(k == KT - 1),
            )
        s["VT"] = VT = chain.tile([128, TT], BF16, tag="VT", name=f"VT{tt}")
        # fold the v-bias here: out(v + bv) = out(v) + bv
        nc.scalar.activation(VT[:, :], vps[:, :], Ident, bias=bv_sb[:, 0:1])

    def emit_prep(tt):
        # token-layout Ek / V (PE transpose + ACT copy), chunk scores, masks
        s = st[tt]
        UT, EkT, VT = s["UT"], s["EkT"], s["VT"]
        trp = ps_tr.tile([128, CPT, 2, C], BF16, tag="tr", name=f"trp{tt}")
        s["ek_toks"], s["v_augs"] = [], []
        # all Ek transposes first: they only need EkT, so the PE isn't
        # stalled behind the VT activation
        for cc in range(CPT):
            nc.tensor.transpose(trp[:, cc, 0, :], EkT[:, ts(cc, C)], id_sb[:, :])
        for cc in range(CPT):
            nc.tensor.transpose(trp[:, cc, 1, :], VT[:, ts(cc, C)], id_sb[:, :])
        for cc in range(CPT):
            ek_tok = small.tile(
                [128, 128], BF16, tag="ektok", bufs=6, name=f"ektok{tt}_{cc}"
            )
            nc.scalar.copy(ek_tok[:, :], trp[:, cc, 0, :])
            s["ek_toks"].append(ek_tok)
            v_aug = small.tile(
                [128, HPC, E + 1], BF16, tag="vaug", bufs=6, name=f"vaug{tt}_{cc}"
            )
            nc.scalar.copy(
                v_aug[:, :, 0:E],
                trp[:, cc, 1, :].rearrange("p (g e) -> p g e", g=HPC),
            )
            nc.gpsimd.memset(v_aug[:, :, E : E + 1], 1.0)
            s["v_augs"].append(v_aug)
        sps = [
            ps_s.tile([128, TT], FP32, tag="s", name=f"sp{tt}_{h}")
            for h in range(HPC)
        ]
        s["smt"] = []
        for cc in range(CPT):
            for h in range(HPC):
                nc.tensor.matmul(
                    sps[h][:, ts(cc, C)],
                    lhsT=EkT[ts(h, E), ts(cc, C)],
                    rhs=UT[ts(h, E), ts(cc, C)],
                    start=True,
                    stop=True,
                    tile_position=(E * h, 0),
                )
            # per-chunk masked scores so chunk 0 can start while later
            # chunks' S matmuls are still in flight
            for h in range(HPC):
                sm = smtp.tile(
                    [128, C], BF16, tag=f"smt{h}", name=f"smt{tt}_{cc}_{h}"
                )
                nc.vector.tensor_mul(sm[:, :], sps[h][:, ts(cc, C)], msk_sb[:, :])
                s["smt"].append(sm)

    def finalize(out_ps, osb, ftt, fcc):
        fc = ftt * CPT + fcc
        last_chunk = fc == NC - 1
        rec = small.tile([128, HPC], FP32, tag="rec", name=f"rec{fc}")
        nc.vector.reciprocal(rec[:, :], out_ps[:, :, E])
        for h in range(HPC):
            # osb = out_ps * rec  (+bv already folded into v)
            nc.scalar.activation(
                osb[:, fcc, ts(h, E)],
                out_ps[:, h, 0:E],
                Copy,
                scale=rec[:, h : h + 1],
            )
            if last_chunk:
                # very last chunk: ship each head half right after its own
                # copy, on separate trigger engines, so the end-of-kernel
                # drain waits on a smaller, earlier transfer
                eng2 = nc.sync if h == 0 else nc.gpsimd
                eng2.dma_start(
                    out_d[ts(fc, C), ts(h, E)], osb[:, fcc, ts(h, E)]
                )
        if last_chunk:
            return
        eng = nc.gpsimd if dma_flip[0] % 2 else nc.sync
        if ftt == NTT - 1:
            # last token tile: ship each chunk as soon as it's done, on
            # alternating trigger engines, to shorten the kernel tail
            dma_flip[0] += 1
            eng.dma_start(out_d[ts(fc, C), :], osb[:, fcc, :])
        elif fcc == CPT - 1:
            dma_flip[0] += 1
            eng.dma_start(
                out_d[ts(ftt, TT), :].rearrange("(cc p) f -> p cc f", p=128),
                osb[:, :, :],
            )

    osb = None

    def emit_chain_chunk(tt, cc):
        nonlocal kv_prev, osb
        s = st[tt]
        UT = s["UT"]
        c = tt * CPT + cc
        ek_tok = s["ek_toks"][cc]
        v_aug = s["v_augs"][cc]
        ob = ps_out.tile([128, 512], FP32, tag="out", name=f"ops{c}")
        out_ps = ob[:, 0 : HPC * (E + 1)].rearrange("p (g e) -> p g e", g=HPC)
        # intra: per head (stationary = masked scores); inter: one matmul
        # over both heads against the block-diagonal KV state
        for h in range(HPC):
            nc.tensor.matmul(
                out_ps[:, h, :],
                lhsT=s["smt"][cc * HPC + h][:, :],
                rhs=v_aug[:, h, :],
                start=(h == 0),
                stop=(c == 0 and h == HPC - 1),
            )
        if c > 0:
            nc.tensor.matmul(
                out_ps[:, :, :],
                lhsT=UT[:, ts(cc, C)],
                rhs=kv_prev[:, :, :],
                start=False,
                stop=True,
            )
        if c < NC - 1:
            # delta into the ping-pong PSUM half, diagonal blocks only
            # (off-blocks stay zero from the one-time memset)
            pp = c % 2
            for h in range(HPC):
                nc.tensor.matmul(
                    dpp[ts(h, E), pp, h, :],
                    lhsT=ek_tok[:, ts(h, E)],
                    rhs=v_aug[:, h, :],
                    start=True,
                    stop=True,
                    tile_position=(0, E * h),
                )
            kv_bf = small.tile(
                [128, HPC, E + 1], BF16, tag="kv", bufs=2, name=f"kvb{c}"
            )
            # single update op on the critical path: state' = delta + state
            nc.vector.tensor_add(
                kv_bf[:, :, :], dpp[:, pp, :, :], kvm[:, 1 - pp, :, :]
            )
            nc.vector.tensor_add(
                kvm[:, pp, :, :], dpp[:, pp, :, :], kvm[:, 1 - pp, :, :]
            )
            kv_prev = kv_bf

        if cc == 0:
            osb = outp.tile([128, CPT, HPC * E], FP32, tag="osb", name=f"osb{tt}")
        finalize(out_ps, osb, tt, cc)

    # ---- software-pipelined emission: the next tile's projection k-slices
    # are woven between the chain chunks so the PE's in-order stream always
    # has independent work queued behind each cross-engine dependency of
    # the sequential state chain.
    for k in range(KT):
        emit_qk_slice(0, k)
    emit_act(0)
    emit_v(0)
    emit_prep(0)
    for tt in range(NTT):
        if tt < NTT - 1:
            nxt = tt + 1

            def qk(lo, hi, a=False):
                def go():
                    for k in range(lo, hi):
                        emit_qk_slice(nxt, k)
                    if a:
                        emit_act(nxt)
                return go

            def vprep():
                def go():
                    emit_v(nxt)
                    emit_prep(nxt)
                return go

            slices = [None, qk(0, 4), qk(4, 8, a=True), vprep()]
        else:
            slices = [None] * CPT
        for cc in range(CPT):
            emit_chain_chunk(tt, cc)
            if slices[cc] is not None:
                slices[cc]()


def build_nc():
    nc = bacc.Bacc(
        "TRN2",
        target_bir_lowering=False,
        debug=False,
        enable_asserts=False,
        num_devices=NCORES,
    )
    mega_d = nc.dram_tensor(
        "mega", [128, 128 + KT * KSL], BF16, kind="ExternalInput"
    ).ap()
    cf_d = nc.dram_tensor("cf", [128, 3], FP32, kind="ExternalInput").ap()
    msk_d = nc.dram_tensor("msk", [128, C], BF16, kind="ExternalInput").ap()
    xt1_d = nc.dram_tensor("xt1", [128, KT * TT], BF16, kind="ExternalInput").ap()
    xt23_d = nc.dram_tensor(
        "xt23", [128, 2 * KT * TT], BF16, kind="ExternalInput"
    ).ap()
    out_d = nc.dram_tensor("out", [N, HPC * E], FP32, kind="ExternalOutput").ap()
    io = (mega_d, cf_d, msk_d, xt1_d, xt23_d, out_d)
    with tile.TileContext(nc) as tc:
        _emit(tc, io)
    nc.compile()
    return nc


def host_inputs(x, W_qvk, b_qvk):
    """Full inputs -> per-core in_maps (host-side shard + transpose)."""
    import ml_dtypes

    x = np.asarray(x, dtype=np.float32).reshape(N, D)
    W = np.asarray(W_qvk, dtype=np.float32)
    b = np.asarray(b_qvk, dtype=np.float32)
    xt = x.T.astype(ml_dtypes.bfloat16)  # (D, N)

    def pack(a):  # (D, M) -> (128, KT*M), partition-contiguous
        kt, m = a.shape[0] // 128, a.shape[1]
        return np.ascontiguousarray(
            a.reshape(kt, 128, m).transpose(1, 0, 2).reshape(128, kt * m)
        )

    xtp = [pack(xt[:, tt * TT : (tt + 1) * TT]) for tt in range(NTT)]
    xt1 = xtp[1]
    xt23 = np.ascontiguousarray(np.concatenate([xtp[2], xtp[3]], axis=1))
    ident = np.eye(128, dtype=ml_dtypes.bfloat16)

    tri = np.tril(np.ones((C, C), dtype=np.float32))  # [i, j] valid j<=i
    mask = np.ascontiguousarray(tri.T.astype(ml_dtypes.bfloat16))  # [j, i]

    in_maps = []
    for core in range(NCORES):
        heads = [HPC * core + i for i in range(HPC)]
        # torch.chunk order in reference: q, v, k
        qcols = np.concatenate([np.arange(E * h, E * h + E) for h in heads])
        vcols = qcols + D
        kcols = qcols + 2 * D
        Wc = np.concatenate(
            [W[:, qcols], W[:, kcols], W[:, vcols]], axis=1
        ).astype(ml_dtypes.bfloat16)  # (D, 384)
        # mega: [ident | (Wq|Wk|Wv | xt0_k) x 8]
        parts = [ident]
        for k in range(KT):
            parts.append(Wc[k * 128 : (k + 1) * 128, :])
            parts.append(xtp[0][:, k * TT : (k + 1) * TT])
        mega = np.ascontiguousarray(np.concatenate(parts, axis=1))
        bq = b[qcols].reshape(128, 1)
        bk = b[kcols].reshape(128, 1)
        bv = b[vcols].reshape(128, 1)
        cf = np.ascontiguousarray(
            np.concatenate([bq, bk, bv], axis=1, dtype=np.float32)
        )
        in_maps.append(dict(mega=mega, cf=cf, msk=mask, xt1=xt1, xt23=xt23))
    return in_maps


_CACHE = {}


def kernel(x, W_qvk, b_qvk, head_num):
    assert int(np.asarray(head_num)) == H
    if "nc" not in _CACHE:
        _CACHE["nc"] = build_nc()
    nc = _CACHE["nc"]
    in_maps = host_inputs(x, W_qvk, b_qvk)
    from concourse.bass_utils import run_bass_kernel_spmd

    res = run_bass_kernel_spmd(
        nc,
        in_maps,
        core_ids=list(range(NCORES)),
        trace=bool(int(os.environ.get("KERNEL_TRACE", "0"))),
    )
    _CACHE["last_result"] = res
    out = np.concatenate([r["out"] for r in res.results], axis=1)
    return out.reshape(B, N, D).astype(np.float32)


# revision 16
# speedup vs baseline: 1.0251x; 1.0251x over previous
"""Multi-head causal linear attention (B=1, N=2048, D=1024, H=16) on 8 trn2 cores.

Math: reference computes, per head (e=64):
    q = softmax(q_raw, -1) * e**-0.5 ;  k = exp(k_raw)
    out_n = (q_n . KV_n) / (q_n . (kcum_n + EPS)),  KV_n = sum_{j<=n} k_j v_j^T
Because both numerator and denominator are linear in q_n, the softmax
normalization and the e**-0.5 scale cancel exactly; only u = exp(q_raw)
matters.  The EPS term contributes <1e-6 relative and is dropped.  The
v-bias is folded into the value vectors (v' = v + b_v): since the
attention weights sum to 1, out(v') = out(v) + b_v exactly.

Per-core work (head-parallel, 2 heads/core):
  * k-outer q/k projection: per 128-wide d-slice, two matmuls (W
    stationary, x^T moving) gated on that slice's [W | x] DMA, so the
    input stream paces the first token tile instead of stalling the PE
    behind a monolithic transfer.  v projection runs k-inner right
    after, reusing the q PSUM bank.
  * Chunked causal linear attention (chunk=128): intra (masked QK^T V)
    + inter (running KV state) recurrence.  The KV state is kept
    block-diagonal over the 2 heads (off-blocks zeroed once) so inter
    is a single matmul over both heads and the per-chunk state update
    is one vector add.
"""

import os
from contextlib import ExitStack

import numpy as np

import concourse.bass as bass
import concourse.mybir as mybir
import concourse.tile as tile
from concourse import bacc
from concourse._compat import with_exitstack
from concourse.bass import ts

FP32 = mybir.dt.float32
BF16 = mybir.dt.bfloat16

B, N, D, H = 1, 2048, 1024, 16
E = D // H          # 64 head dim
NCORES = 8
HPC = H // NCORES   # 2 heads per core
KT = D // 128       # 8 contraction (d) slices
TT = 512            # token tile (projection granularity)
NTT = N // TT       # 4
C = 128             # chunk (tokens) for the causal recurrence
CPT = TT // C       # 4 chunks per token tile
NC = N // C         # 16 chunks total
WB = 3 * 128        # W columns per k-slice (q|k|v blocks)
KSL = WB + TT       # mega cols per k-slice: [Wq|Wk|Wv|X0]
NJUNK = 6           # PE warm-up matmuls (bridge DMA arm latency ~2.5us)

Exp = mybir.ActivationFunctionType.Exp
Copy = mybir.ActivationFunctionType.Copy
Ident = mybir.ActivationFunctionType.Identity


@with_exitstack
def _emit(ctx: ExitStack, tc, io):
    nc = tc.nc
    mega_d, cf_d, msk_d, xt1_d, xt23_d, out_d = io

    const = ctx.enter_context(tc.tile_pool(name="const", bufs=1))
    chain = ctx.enter_context(tc.tile_pool(name="chain", bufs=2))
    smtp = ctx.enter_context(tc.tile_pool(name="smtp", bufs=4))
    small = ctx.enter_context(tc.tile_pool(name="small", bufs=3))
    outp = ctx.enter_context(tc.tile_pool(name="outp", bufs=3))
    pproj = ctx.enter_context(tc.tile_pool(name="pproj", bufs=1, space="PSUM"))
    ps_s = ctx.enter_context(tc.tile_pool(name="ps_s", bufs=2, space="PSUM"))
    ps_tr = ctx.enter_context(tc.tile_pool(name="ps_tr", bufs=1, space="PSUM"))
    ps_out = ctx.enter_context(tc.tile_pool(name="ps_out", bufs=2, space="PSUM"))
    ps_dp = ctx.enter_context(tc.tile_pool(name="ps_dp", bufs=1, space="PSUM"))

    # ---- persistent SBUF ----
    # mega: [ ident 128 | (Wq|Wk|Wv 384 | xt0 512) x 8 ]
    mega_sb = const.tile([128, 128 + KT * KSL], BF16)
    cf_sb = const.tile([128, 3], FP32)             # [bq|bk|bv]
    msk_sb = const.tile([128, C], BF16)            # causal mask [j, i], 1 iff j<=i
    xtr_sb = const.tile([128, (NTT - 1) * KT * TT], BF16)  # xt tt=1..3, (tt k t)

    id_sb = mega_sb[:, 0:128]
    bq_sb = cf_sb[:, 0:1]
    bk_sb = cf_sb[:, 1:2]
    bv_sb = cf_sb[:, 2:3]

    def w_ap(k, f):
        base = 128 + k * KSL + f * 128
        return mega_sb[:, base : base + 128]

    def xt_ap(tt, k):
        if tt == 0:
            base = 128 + k * KSL + WB
            return mega_sb[:, base : base + TT]
        base = (tt - 1) * KT * TT + k * TT
        return xtr_sb[:, base : base + TT]

    # block-diagonal KV state: delta PSUM ping-pong + fp32 master
    # ping-pong, all with off-diagonal head blocks zeroed once.
    dpp_bank = ps_dp.tile([128, 512], FP32)
    dpp = dpp_bank[:, 0 : 2 * HPC * (E + 1)].rearrange(
        "p (a g e) -> p a g e", a=2, g=HPC
    )
    nc.vector.memset(dpp[:, :, :, :], 0.0)
    kvm = const.tile([128, 2, HPC, E + 1], FP32)
    nc.vector.memset(kvm[:, :, :, :], 0.0)

    # PE warm-up bridge: junk matmuls on zeros keep the HAM activity window
    # busy while inputs stream in, so real matmuls start at 2.4 GHz.
    scratch = const.tile([128, TT], BF16)
    nc.gpsimd.memset(scratch[:, :], 0.0)
    junk_ps = ps_s.tile([128, TT], FP32, tag="s", name="junk")

    def junk(n):
        for j in range(n):
            nc.tensor.matmul(
                junk_ps[:, :],
                lhsT=scratch[:, 0:128],
                rhs=scratch[:, :],
                start=True,
                stop=True,
            )

    junk(NJUNK)

    # input DMAs, strictly in consumption order on one engine so the HBM
    # stream is never stolen by data needed later. k-slice-pair
    # granularity gates the tile-0 projection.
    M0 = 128  # ident rides with the first pair
    for kp in range(KT // 2):
        lo = (M0 if kp else 0) + kp * 2 * KSL
        hi = M0 + (kp + 1) * 2 * KSL
        nc.sync.dma_start(mega_sb[:, lo:hi], mega_d[:, lo:hi])
        if kp == 1:
            nc.sync.dma_start(cf_sb[:, :], cf_d[:, :])
            nc.sync.dma_start(msk_sb[:, :], msk_d[:, :])
    T3 = 3 * TT
    nc.sync.dma_start(xtr_sb[:, 0:T3], xt1_d[:, 0:T3])
    nc.sync.dma_start(xtr_sb[:, T3 : 2 * T3], xt1_d[:, T3 : 2 * T3])
    nc.sync.dma_start(xtr_sb[:, 2 * T3 : KT * TT], xt1_d[:, 2 * T3 : KT * TT])
    nc.sync.dma_start(
        xtr_sb[:, KT * TT : 2 * KT * TT], xt23_d[:, 0 : KT * TT]
    )
    nc.sync.dma_start(
        xtr_sb[:, 2 * KT * TT : 3 * KT * TT], xt23_d[:, KT * TT : 2 * KT * TT]
    )

    kv_prev = None   # bf16 block-diagonal [128, HPC, E+1] (matmul operand)
    dma_flip = [0]

    st = [dict() for _ in range(NTT)]

    def emit_qk_slice(tt, k):
        s = st[tt]
        if k == 0:
            s["qps"] = pproj.tile([128, TT], FP32, tag="q", name=f"qps{tt}")
            s["kps"] = pproj.tile([128, TT], FP32, tag="k", name=f"kps{tt}")
        first, last = k == 0, k == KT - 1
        nc.tensor.matmul(
            s["kps"][:, :], lhsT=w_ap(k, 1), rhs=xt_ap(tt, k),
            start=first, stop=last,
        )
        nc.tensor.matmul(
            s["qps"][:, :], lhsT=w_ap(k, 0), rhs=xt_ap(tt, k),
            start=first, stop=last,
        )

    def emit_act(tt):
        s = st[tt]
        s["EkT"] = EkT = chain.tile([128, TT], BF16, tag="EkT", name=f"EkT{tt}")
        nc.scalar.activation(EkT[:, :], s["kps"][:, :], Exp, bias=bk_sb[:, 0:1])
        s["UT"] = UT = chain.tile([128, TT], BF16, tag="UT", name=f"UT{tt}")
        nc.scalar.activation(UT[:, :], s["qps"][:, :], Exp, bias=bq_sb[:, 0:1])

    def emit_v(tt):
        # v projection k-inner, reusing the q PSUM bank (freed by UT act)
        s = st[tt]
        vps = pproj.tile([128, TT], FP32, tag="q", name=f"vps{tt}")
        for k in range(KT):
            nc.tensor.matmul(
                vps[:, :], lhsT=w_ap(k, 2), rhs=xt_ap(tt, k),
                start=(k == 0), stop=(k == KT - 1),
            )
        s["VT"] = VT = chain.tile([128, TT], BF16, tag="VT", name=f"VT{tt}")
        # fold the v-bias here: out(v + bv) = out(v) + bv
        nc.scalar.activation(VT[:, :], vps[:, :], Ident, bias=bv_sb[:, 0:1])

    def emit_prep(tt):
        # token-layout Ek / V (PE transpose + ACT copy), chunk scores, masks
        s = st[tt]
        UT, EkT, VT = s["UT"], s["EkT"], s["VT"]
        trp = ps_tr.tile([128, CPT, 2, C], BF16, tag="tr", name=f"trp{tt}")
        s["ek_toks"], s["v_augs"] = [], []
        # all Ek transposes first: they only need EkT, so the PE isn't
        # stalled behind the VT activation
        for cc in range(CPT):
            nc.tensor.transpose(trp[:, cc, 0, :], EkT[:, ts(cc, C)], id_sb[:, :])
        for cc in range(CPT):
            nc.tensor.transpose(trp[:, cc, 1, :], VT[:, ts(cc, C)], id_sb[:, :])
        for cc in range(CPT):
            ek_tok = small.tile(
                [128, 128], BF16, tag="ektok", bufs=6, name=f"ektok{tt}_{cc}"
            )
            nc.scalar.copy(ek_tok[:, :], trp[:, cc, 0, :])
            s["ek_toks"].append(ek_tok)
            v_aug = small.tile(
                [128, HPC, E + 1], BF16, tag="vaug", bufs=6, name=f"vaug{tt}_{cc}"
            )
            nc.scalar.copy(
                v_aug[:, :, 0:E],
                trp[:, cc, 1, :].rearrange("p (g e) -> p g e", g=HPC),
            )
            nc.gpsimd.memset(v_aug[:, :, E : E + 1], 1.0)
            s["v_augs"].append(v_aug)
        sps = [
            ps_s.tile([128, TT], FP32, tag="s", name=f"sp{tt}_{h}")
            for h in range(HPC)
        ]
        s["smt"] = []
        for cc in range(CPT):
            for h in range(HPC):
                nc.tensor.matmul(
                    sps[h][:, ts(cc, C)],
                    lhsT=EkT[ts(h, E), ts(cc, C)],
                    rhs=UT[ts(h, E), ts(cc, C)],
                    start=True,
                    stop=True,
                    tile_position=(E * h, 0),
                )
            # per-chunk masked scores so chunk 0 can start while later
            # chunks' S matmuls are still in flight
            for h in range(HPC):
                sm = smtp.tile(
                    [128, C], BF16, tag=f"smt{h}", name=f"smt{tt}_{cc}_{h}"
                )
                nc.vector.tensor_mul(sm[:, :], sps[h][:, ts(cc, C)], msk_sb[:, :])
                s["smt"].append(sm)

    def finalize(out_ps, osb, ftt, fcc):
        fc = ftt * CPT + fcc
        last_chunk = fc == NC - 1
        rec = small.tile([128, HPC], FP32, tag="rec", name=f"rec{fc}")
        nc.vector.reciprocal(rec[:, :], out_ps[:, :, E])
        for h in range(HPC):
            # osb = out_ps * rec  (+bv already folded into v)
            nc.scalar.activation(
                osb[:, fcc, ts(h, E)],
                out_ps[:, h, 0:E],
                Copy,
                scale=rec[:, h : h + 1],
            )
            if last_chunk:
                # very last chunk: ship each head half right after its own
                # copy, on separate trigger engines, so the end-of-kernel
                # drain waits on a smaller, earlier transfer
                eng2 = nc.sync if h == 0 else nc.gpsimd
                eng2.dma_start(
                    out_d[ts(fc, C), ts(h, E)], osb[:, fcc, ts(h, E)]
                )
        if last_chunk:
            return
        eng = nc.gpsimd if dma_flip[0] % 2 else nc.sync
        if ftt == NTT - 1:
            # last token tile: ship each chunk as soon as it's done, on
            # alternating trigger engines, to shorten the kernel tail
            dma_flip[0] += 1
            eng.dma_start(out_d[ts(fc, C), :], osb[:, fcc, :])
        elif fcc == CPT - 1:
            dma_flip[0] += 1
            eng.dma_start(
                out_d[ts(ftt, TT), :].rearrange("(cc p) f -> p cc f", p=128),
                osb[:, :, :],
            )

    osb = None

    def emit_chain_chunk(tt, cc):
        nonlocal kv_prev, osb
        s = st[tt]
        UT = s["UT"]
        c = tt * CPT + cc
        ek_tok = s["ek_toks"][cc]
        v_aug = s["v_augs"][cc]
        ob = ps_out.tile([128, 512], FP32, tag="out", name=f"ops{c}")
        out_ps = ob[:, 0 : HPC * (E + 1)].rearrange("p (g e) -> p g e", g=HPC)
        # intra: per head (stationary = masked scores); inter: one matmul
        # over both heads against the block-diagonal KV state
        for h in range(HPC):
            nc.tensor.matmul(
                out_ps[:, h, :],
                lhsT=s["smt"][cc * HPC + h][:, :],
                rhs=v_aug[:, h, :],
                start=(h == 0),
                stop=(c == 0 and h == HPC - 1),
            )
        if c > 0:
            nc.tensor.matmul(
                out_ps[:, :, :],
                lhsT=UT[:, ts(cc, C)],
                rhs=kv_prev[:, :, :],
                start=False,
                stop=True,
            )
        if c < NC - 1:
            # delta into the ping-pong PSUM half, diagonal blocks only
            # (off-blocks stay zero from the one-time memset)
            pp = c % 2
            for h in range(HPC):
                nc.tensor.matmul(
                    dpp[ts(h, E), pp, h, :],
                    lhsT=ek_tok[:, ts(h, E)],
                    rhs=v_aug[:, h, :],
                    start=True,
                    stop=True,
                    tile_position=(0, E * h),
                )
            kv_bf = small.tile(
                [128, HPC, E + 1], BF16, tag="kv", bufs=2, name=f"kvb{c}"
            )
            # single update op on the critical path: state' = delta + state
            nc.vector.tensor_add(
                kv_bf[:, :, :], dpp[:, pp, :, :], kvm[:, 1 - pp, :, :]
            )
            nc.vector.tensor_add(
                kvm[:, pp, :, :], dpp[:, pp, :, :], kvm[:, 1 - pp, :, :]
            )
            kv_prev = kv_bf

        if cc == 0:
            osb = outp.tile([128, CPT, HPC * E], FP32, tag="osb", name=f"osb{tt}")
        finalize(out_ps, osb, tt, cc)

    # ---- software-pipelined emission: the next tile's projection k-slices
    # are woven between the chain chunks so the PE's in-order stream always
    # has independent work queued behind each cross-engine dependency of
    # the sequential state chain.
    # tile 0 is DMA-paced: junk-fill the per-pair stream gaps so the PE
    # never idles long enough for the HAM window to keep it throttled.
    for k in range(KT):
        emit_qk_slice(0, k)
        if k % 2 == 1:
            junk(2)
    emit_act(0)
    emit_v(0)
    emit_prep(0)
    for tt in range(NTT):
        if tt < NTT - 1:
            nxt = tt + 1

            def qk(lo, hi, a=False):
                def go():
                    for k in range(lo, hi):
                        emit_qk_slice(nxt, k)
                    if a:
                        emit_act(nxt)
                return go

            def vprep():
                def go():
                    emit_v(nxt)
                    emit_prep(nxt)
                return go

            slices = [qk(0, 3), qk(3, 6), qk(6, 8, a=True), vprep()]
        else:
            slices = [None] * CPT
        for cc in range(CPT):
            emit_chain_chunk(tt, cc)
            if slices[cc] is not None:
                slices[cc]()


def build_nc():
    nc = bacc.Bacc(
        "TRN2",
        target_bir_lowering=False,
        debug=False,
        enable_asserts=False,
        num_devices=NCORES,
    )
    mega_d = nc.dram_tensor(
        "mega", [128, 128 + KT * KSL], BF16, kind="ExternalInput"
    ).ap()
    cf_d = nc.dram_tensor("cf", [128, 3], FP32, kind="ExternalInput").ap()
    msk_d = nc.dram_tensor("msk", [128, C], BF16, kind="ExternalInput").ap()
    xt1_d = nc.dram_tensor("xt1", [128, KT * TT], BF16, kind="ExternalInput").ap()
    xt23_d = nc.dram_tensor(
        "xt23", [128, 2 * KT * TT], BF16, kind="ExternalInput"
    ).ap()
    out_d = nc.dram_tensor("out", [N, HPC * E], FP32, kind="ExternalOutput").ap()
    io = (mega_d, cf_d, msk_d, xt1_d, xt23_d, out_d)
    with tile.TileContext(nc) as tc:
        _emit(tc, io)
    nc.compile()
    return nc


def host_inputs(x, W_qvk, b_qvk):
    """Full inputs -> per-core in_maps (host-side shard + transpose)."""
    import ml_dtypes

    x = np.asarray(x, dtype=np.float32).reshape(N, D)
    W = np.asarray(W_qvk, dtype=np.float32)
    b = np.asarray(b_qvk, dtype=np.float32)
    xt = x.T.astype(ml_dtypes.bfloat16)  # (D, N)

    def pack(a):  # (D, M) -> (128, KT*M), partition-contiguous
        kt, m = a.shape[0] // 128, a.shape[1]
        return np.ascontiguousarray(
            a.reshape(kt, 128, m).transpose(1, 0, 2).reshape(128, kt * m)
        )

    xtp = [pack(xt[:, tt * TT : (tt + 1) * TT]) for tt in range(NTT)]
    xt1 = xtp[1]
    xt23 = np.ascontiguousarray(np.concatenate([xtp[2], xtp[3]], axis=1))
    ident = np.eye(128, dtype=ml_dtypes.bfloat16)

    tri = np.tril(np.ones((C, C), dtype=np.float32))  # [i, j] valid j<=i
    mask = np.ascontiguousarray(tri.T.astype(ml_dtypes.bfloat16))  # [j, i]

    in_maps = []
    for core in range(NCORES):
        heads = [HPC * core + i for i in range(HPC)]
        # torch.chunk order in reference: q, v, k
        qcols = np.concatenate([np.arange(E * h, E * h + E) for h in heads])
        vcols = qcols + D
        kcols = qcols + 2 * D
        Wc = np.concatenate(
            [W[:, qcols], W[:, kcols], W[:, vcols]], axis=1
        ).astype(ml_dtypes.bfloat16)  # (D, 384)
        # mega: [ident | (Wq|Wk|Wv | xt0_k) x 8]
        parts = [ident]
        for k in range(KT):
            parts.append(Wc[k * 128 : (k + 1) * 128, :])
            parts.append(xtp[0][:, k * TT : (k + 1) * TT])
        mega = np.ascontiguousarray(np.concatenate(parts, axis=1))
        bq = b[qcols].reshape(128, 1)
        bk = b[kcols].reshape(128, 1)
        bv = b[vcols].reshape(128, 1)
        cf = np.ascontiguousarray(
            np.concatenate([bq, bk, bv], axis=1, dtype=np.float32)
        )
        in_maps.append(dict(mega=mega, cf=cf, msk=mask, xt1=xt1, xt23=xt23))
    return in_maps


_CACHE = {}


def kernel(x, W_qvk, b_qvk, head_num):
    assert int(np.asarray(head_num)) == H
    if "nc" not in _CACHE:
        _CACHE["nc"] = build_nc()
    nc = _CACHE["nc"]
    in_maps = host_inputs(x, W_qvk, b_qvk)
    from concourse.bass_utils import run_bass_kernel_spmd

    res = run_bass_kernel_spmd(
        nc,
        in_maps,
        core_ids=list(range(NCORES)),
        trace=bool(int(os.environ.get("KERNEL_TRACE", "0"))),
    )
    _CACHE["last_result"] = res
    out = np.concatenate([r["out"] for r in res.results], axis=1)
    return out.reshape(B, N, D).astype(np.float32)


# revision 20
# speedup vs baseline: 1.1163x; 1.0889x over previous
"""Multi-head causal linear attention (B=1, N=2048, D=1024, H=16) on 8 trn2 cores.

Math: reference computes, per head (e=64):
    q = softmax(q_raw, -1) * e**-0.5 ;  k = exp(k_raw)
    out_n = (q_n . KV_n) / (q_n . (kcum_n + EPS)),  KV_n = sum_{j<=n} k_j v_j^T
Because both numerator and denominator are linear in q_n, the softmax
normalization and the e**-0.5 scale cancel exactly; only u = exp(q_raw)
matters.  The EPS term contributes <1e-6 relative and is dropped.  The
v-bias is folded into the value vectors (v' = v + b_v): since the
attention weights sum to 1, out(v') = out(v) + b_v exactly.

Per-core work (head-parallel, 2 heads/core):
  * k-outer q/k projection: per 128-wide d-slice, two matmuls (W
    stationary, x^T moving) gated on that slice's [W | x] DMA, so the
    input stream paces the first token tile instead of stalling the PE
    behind a monolithic transfer.  v projection runs k-inner right
    after, reusing the q PSUM bank.
  * Chunked causal linear attention (chunk=128): intra (masked QK^T V)
    + inter (running KV state) recurrence.  The KV state is kept
    block-diagonal over the 2 heads (off-blocks zeroed once) so inter
    is a single matmul over both heads and the per-chunk state update
    is one vector add.
"""

import os
from contextlib import ExitStack

import numpy as np

import concourse.bass as bass
import concourse.mybir as mybir
import concourse.tile as tile
from concourse import bacc
from concourse._compat import with_exitstack
from concourse.bass import ts

FP32 = mybir.dt.float32
BF16 = mybir.dt.bfloat16

B, N, D, H = 1, 2048, 1024, 16
E = D // H          # 64 head dim
NCORES = 8
HPC = H // NCORES   # 2 heads per core
KT = D // 128       # 8 contraction (d) slices
TT = 512            # token tile (projection granularity)
NTT = N // TT       # 4
C = 128             # chunk (tokens) for the causal recurrence
CPT = TT // C       # 4 chunks per token tile
NC = N // C         # 16 chunks total
WB = 3 * 128        # W columns per k-slice (q|k|v blocks)
KSL = WB + TT       # mega cols per k-slice: [Wq|Wk|Wv|X0]
NJUNK = 6           # PE warm-up matmuls (bridge DMA arm latency ~2.5us)

Exp = mybir.ActivationFunctionType.Exp
Copy = mybir.ActivationFunctionType.Copy
Ident = mybir.ActivationFunctionType.Identity


@with_exitstack
def _emit(ctx: ExitStack, tc, io):
    nc = tc.nc
    mega_d, cf_d, msk_d, xt1_d, xt23_d, out_d = io

    const = ctx.enter_context(tc.tile_pool(name="const", bufs=1))
    chain = ctx.enter_context(tc.tile_pool(name="chain", bufs=2))
    smtp = ctx.enter_context(tc.tile_pool(name="smtp", bufs=4))
    small = ctx.enter_context(tc.tile_pool(name="small", bufs=3))
    outp = ctx.enter_context(tc.tile_pool(name="outp", bufs=3))
    pproj = ctx.enter_context(tc.tile_pool(name="pproj", bufs=1, space="PSUM"))
    ps_s = ctx.enter_context(tc.tile_pool(name="ps_s", bufs=2, space="PSUM"))
    ps_tr = ctx.enter_context(tc.tile_pool(name="ps_tr", bufs=1, space="PSUM"))
    ps_out = ctx.enter_context(tc.tile_pool(name="ps_out", bufs=2, space="PSUM"))
    ps_dp = ctx.enter_context(tc.tile_pool(name="ps_dp", bufs=1, space="PSUM"))

    # ---- persistent SBUF ----
    # mega: [ ident 128 | (Wq|Wk|Wv 384 | xt0 512) x 8 ]
    mega_sb = const.tile([128, 128 + KT * KSL], BF16)
    cf_sb = const.tile([128, 3], FP32)             # [bq|bk|bv]
    msk_sb = const.tile([128, C], BF16)            # causal mask [j, i], 1 iff j<=i
    xtr_sb = const.tile([128, (NTT - 1) * KT * TT], BF16)  # xt tt=1..3, (tt k t)

    id_sb = mega_sb[:, 0:128]
    bq_sb = cf_sb[:, 0:1]
    bk_sb = cf_sb[:, 1:2]
    bv_sb = cf_sb[:, 2:3]

    def w_ap(k, f):
        base = 128 + k * KSL + f * 128
        return mega_sb[:, base : base + 128]

    def xt_ap(tt, k):
        if tt == 0:
            base = 128 + k * KSL + WB
            return mega_sb[:, base : base + TT]
        base = (tt - 1) * KT * TT + k * TT
        return xtr_sb[:, base : base + TT]

    # block-diagonal KV state: delta PSUM ping-pong + fp32 master
    # ping-pong, all with off-diagonal head blocks zeroed once.
    dpp_bank = ps_dp.tile([128, 512], FP32)
    dpp = dpp_bank[:, 0 : 2 * HPC * (E + 1)].rearrange(
        "p (a g e) -> p a g e", a=2, g=HPC
    )
    nc.vector.memset(dpp[:, :, :, :], 0.0)
    kvm = const.tile([128, 2, HPC, E + 1], FP32)
    nc.vector.memset(kvm[:, :, :, :], 0.0)

    # PE warm-up bridge: junk matmuls on zeros keep the HAM activity window
    # busy while inputs stream in, so real matmuls start at 2.4 GHz.
    scratch = const.tile([128, TT], BF16)
    nc.gpsimd.memset(scratch[:, :], 0.0)
    junk_ps = ps_s.tile([128, TT], FP32, tag="s", name="junk")

    def junk(n):
        for j in range(n):
            nc.tensor.matmul(
                junk_ps[:, :],
                lhsT=scratch[:, 0:128],
                rhs=scratch[:, :],
                start=True,
                stop=True,
            )

    junk(NJUNK)

    # input DMAs, strictly in consumption order on one engine so the HBM
    # stream is never stolen by data needed later. k-slice-pair
    # granularity gates the tile-0 projection.
    M0 = 128  # ident rides with the first pair
    for kp in range(KT // 2):
        lo = (M0 if kp else 0) + kp * 2 * KSL
        hi = M0 + (kp + 1) * 2 * KSL
        nc.sync.dma_start(mega_sb[:, lo:hi], mega_d[:, lo:hi])
        if kp == 1:
            nc.sync.dma_start(cf_sb[:, :], cf_d[:, :])
            nc.sync.dma_start(msk_sb[:, :], msk_d[:, :])
    T3 = 3 * TT
    nc.sync.dma_start(xtr_sb[:, 0:T3], xt1_d[:, 0:T3])
    nc.sync.dma_start(xtr_sb[:, T3 : 2 * T3], xt1_d[:, T3 : 2 * T3])
    nc.sync.dma_start(xtr_sb[:, 2 * T3 : KT * TT], xt1_d[:, 2 * T3 : KT * TT])
    nc.sync.dma_start(
        xtr_sb[:, KT * TT : 2 * KT * TT], xt23_d[:, 0 : KT * TT]
    )
    nc.sync.dma_start(
        xtr_sb[:, 2 * KT * TT : 3 * KT * TT], xt23_d[:, KT * TT : 2 * KT * TT]
    )

    kv_prev = None   # bf16 block-diagonal [128, HPC, E+1] (matmul operand)
    dma_flip = [0]

    st = [dict() for _ in range(NTT)]

    def emit_qk_slice(tt, k):
        s = st[tt]
        if k == 0:
            s["qps"] = pproj.tile([128, TT], FP32, tag="q", name=f"qps{tt}")
            s["kps"] = pproj.tile([128, TT], FP32, tag="k", name=f"kps{tt}")
        first, last = k == 0, k == KT - 1
        nc.tensor.matmul(
            s["kps"][:, :], lhsT=w_ap(k, 1), rhs=xt_ap(tt, k),
            start=first, stop=last,
        )
        nc.tensor.matmul(
            s["qps"][:, :], lhsT=w_ap(k, 0), rhs=xt_ap(tt, k),
            start=first, stop=last,
        )

    def emit_act(tt):
        s = st[tt]
        s["EkT"] = EkT = chain.tile([128, TT], BF16, tag="EkT", name=f"EkT{tt}")
        nc.scalar.activation(EkT[:, :], s["kps"][:, :], Exp, bias=bk_sb[:, 0:1])
        s["UT"] = UT = chain.tile([128, TT], BF16, tag="UT", name=f"UT{tt}")
        nc.scalar.activation(UT[:, :], s["qps"][:, :], Exp, bias=bq_sb[:, 0:1])

    def emit_v(tt):
        # v projection k-inner, reusing the S PSUM ring (free once the
        # previous tile's masked scores are built) so it doesn't wait on
        # the UT activation
        s = st[tt]
        vps = ps_s.tile([128, TT], FP32, tag="s", name=f"vps{tt}")
        for k in range(KT):
            nc.tensor.matmul(
                vps[:, :], lhsT=w_ap(k, 2), rhs=xt_ap(tt, k),
                start=(k == 0), stop=(k == KT - 1),
            )
        s["VT"] = VT = chain.tile([128, TT], BF16, tag="VT", name=f"VT{tt}")
        # fold the v-bias here: out(v + bv) = out(v) + bv
        nc.scalar.activation(VT[:, :], vps[:, :], Ident, bias=bv_sb[:, 0:1])

    def emit_prep(tt):
        # token-layout Ek / V (PE transpose + ACT copy), chunk scores, masks
        s = st[tt]
        UT, EkT, VT = s["UT"], s["EkT"], s["VT"]
        trp = ps_tr.tile([128, CPT, 2, C], BF16, tag="tr", name=f"trp{tt}")
        s["ek_toks"], s["v_augs"] = [], []
        # all Ek transposes first: they only need EkT, so the PE isn't
        # stalled behind the VT activation
        for cc in range(CPT):
            nc.tensor.transpose(trp[:, cc, 0, :], EkT[:, ts(cc, C)], id_sb[:, :])
        for cc in range(CPT):
            nc.tensor.transpose(trp[:, cc, 1, :], VT[:, ts(cc, C)], id_sb[:, :])
        for cc in range(CPT):
            ek_tok = small.tile(
                [128, 128], BF16, tag="ektok", bufs=6, name=f"ektok{tt}_{cc}"
            )
            nc.scalar.copy(ek_tok[:, :], trp[:, cc, 0, :])
            s["ek_toks"].append(ek_tok)
            v_aug = small.tile(
                [128, HPC, E + 1], BF16, tag="vaug", bufs=6, name=f"vaug{tt}_{cc}"
            )
            nc.scalar.copy(
                v_aug[:, :, 0:E],
                trp[:, cc, 1, :].rearrange("p (g e) -> p g e", g=HPC),
            )
            nc.gpsimd.memset(v_aug[:, :, E : E + 1], 1.0)
            s["v_augs"].append(v_aug)
        sps = [
            ps_s.tile([128, TT], FP32, tag="s", name=f"sp{tt}_{h}")
            for h in range(HPC)
        ]
        s["smt"] = []
        for cc in range(CPT):
            for h in range(HPC):
                nc.tensor.matmul(
                    sps[h][:, ts(cc, C)],
                    lhsT=EkT[ts(h, E), ts(cc, C)],
                    rhs=UT[ts(h, E), ts(cc, C)],
                    start=True,
                    stop=True,
                    tile_position=(E * h, 0),
                )
            # per-chunk masked scores so chunk 0 can start while later
            # chunks' S matmuls are still in flight
            for h in range(HPC):
                sm = smtp.tile(
                    [128, C], BF16, tag=f"smt{h}", name=f"smt{tt}_{cc}_{h}"
                )
                nc.vector.tensor_mul(sm[:, :], sps[h][:, ts(cc, C)], msk_sb[:, :])
                s["smt"].append(sm)

    def finalize(out_ps, osb, ftt, fcc):
        fc = ftt * CPT + fcc
        last_chunk = fc == NC - 1
        rec = small.tile([128, HPC], FP32, tag="rec", name=f"rec{fc}")
        nc.vector.reciprocal(rec[:, :], out_ps[:, :, E])
        for h in range(HPC):
            # osb = out_ps * rec  (+bv already folded into v); in the
            # kernel tail, split the two copies across engines
            if fc >= NC - 2 and h == 1:
                nc.vector.tensor_scalar_mul(
                    osb[:, fcc, ts(h, E)], out_ps[:, h, 0:E], rec[:, h : h + 1]
                )
            else:
                nc.scalar.activation(
                    osb[:, fcc, ts(h, E)],
                    out_ps[:, h, 0:E],
                    Copy,
                    scale=rec[:, h : h + 1],
                )
            if last_chunk:
                # very last chunk: ship each head half right after its own
                # copy, on separate trigger engines, so the end-of-kernel
                # drain waits on a smaller, earlier transfer
                eng2 = nc.sync if h == 0 else nc.gpsimd
                eng2.dma_start(
                    out_d[ts(fc, C), ts(h, E)], osb[:, fcc, ts(h, E)]
                )
        if last_chunk:
            return
        eng = nc.gpsimd if dma_flip[0] % 2 else nc.sync
        if ftt == NTT - 1:
            # last token tile: ship each chunk as soon as it's done, on
            # alternating trigger engines, to shorten the kernel tail
            dma_flip[0] += 1
            eng.dma_start(out_d[ts(fc, C), :], osb[:, fcc, :])
        elif fcc == CPT - 1:
            dma_flip[0] += 1
            eng.dma_start(
                out_d[ts(ftt, TT), :].rearrange("(cc p) f -> p cc f", p=128),
                osb[:, :, :],
            )

    osb = None

    def emit_chain_chunk(tt, cc):
        nonlocal kv_prev, osb
        s = st[tt]
        UT = s["UT"]
        c = tt * CPT + cc
        ek_tok = s["ek_toks"][cc]
        v_aug = s["v_augs"][cc]
        ob = ps_out.tile([128, 512], FP32, tag="out", name=f"ops{c}")
        out_ps = ob[:, 0 : HPC * (E + 1)].rearrange("p (g e) -> p g e", g=HPC)
        # intra: per head (stationary = masked scores); inter: one matmul
        # over both heads against the block-diagonal KV state
        for h in range(HPC):
            nc.tensor.matmul(
                out_ps[:, h, :],
                lhsT=s["smt"][cc * HPC + h][:, :],
                rhs=v_aug[:, h, :],
                start=(h == 0),
                stop=(c == 0 and h == HPC - 1),
            )
        if c > 0:
            nc.tensor.matmul(
                out_ps[:, :, :],
                lhsT=UT[:, ts(cc, C)],
                rhs=kv_prev[:, :, :],
                start=False,
                stop=True,
            )
        if c < NC - 1:
            # delta into the ping-pong PSUM half, diagonal blocks only
            # (off-blocks stay zero from the one-time memset)
            pp = c % 2
            for h in range(HPC):
                nc.tensor.matmul(
                    dpp[ts(h, E), pp, h, :],
                    lhsT=ek_tok[:, ts(h, E)],
                    rhs=v_aug[:, h, :],
                    start=True,
                    stop=True,
                    tile_position=(0, E * h),
                )
            kv_bf = small.tile(
                [128, HPC, E + 1], BF16, tag="kv", bufs=2, name=f"kvb{c}"
            )
            # single update op on the critical path: state' = delta + state
            nc.vector.tensor_add(
                kv_bf[:, :, :], dpp[:, pp, :, :], kvm[:, 1 - pp, :, :]
            )
            nc.vector.tensor_add(
                kvm[:, pp, :, :], dpp[:, pp, :, :], kvm[:, 1 - pp, :, :]
            )
            kv_prev = kv_bf

        if cc == 0:
            osb = outp.tile([128, CPT, HPC * E], FP32, tag="osb", name=f"osb{tt}")
        finalize(out_ps, osb, tt, cc)

    # ---- software-pipelined emission: the next tile's projection k-slices
    # are woven between the chain chunks so the PE's in-order stream always
    # has independent work queued behind each cross-engine dependency of
    # the sequential state chain.
    # tile 0 is DMA-paced: junk-fill the per-pair stream gaps so the PE
    # never idles long enough for the HAM window to keep it throttled.
    for k in range(KT):
        emit_qk_slice(0, k)
        if k % 2 == 1:
            junk(2)
    emit_act(0)
    emit_v(0)
    emit_prep(0)
    for tt in range(NTT):
        if tt < NTT - 1:
            nxt = tt + 1

            def qk(lo, hi, a=False):
                def go():
                    for k in range(lo, hi):
                        emit_qk_slice(nxt, k)
                    if a:
                        emit_act(nxt)
                return go

            def vprep():
                def go():
                    emit_v(nxt)
                    emit_prep(nxt)
                return go

            slices = [qk(0, 3), qk(3, 6), qk(6, 8, a=True), vprep()]
        else:
            slices = [None] * CPT
        for cc in range(CPT):
            emit_chain_chunk(tt, cc)
            if slices[cc] is not None:
                slices[cc]()


def build_nc():
    nc = bacc.Bacc(
        "TRN2",
        target_bir_lowering=False,
        debug=False,
        enable_asserts=False,
        num_devices=NCORES,
    )
    mega_d = nc.dram_tensor(
        "mega", [128, 128 + KT * KSL], BF16, kind="ExternalInput"
    ).ap()
    cf_d = nc.dram_tensor("cf", [128, 3], FP32, kind="ExternalInput").ap()
    msk_d = nc.dram_tensor("msk", [128, C], BF16, kind="ExternalInput").ap()
    xt1_d = nc.dram_tensor("xt1", [128, KT * TT], BF16, kind="ExternalInput").ap()
    xt23_d = nc.dram_tensor(
        "xt23", [128, 2 * KT * TT], BF16, kind="ExternalInput"
    ).ap()
    out_d = nc.dram_tensor("out", [N, HPC * E], FP32, kind="ExternalOutput").ap()
    io = (mega_d, cf_d, msk_d, xt1_d, xt23_d, out_d)
    with tile.TileContext(nc) as tc:
        _emit(tc, io)
    nc.compile()
    return nc


def host_inputs(x, W_qvk, b_qvk):
    """Full inputs -> per-core in_maps (host-side shard + transpose)."""
    import ml_dtypes

    x = np.asarray(x, dtype=np.float32).reshape(N, D)
    W = np.asarray(W_qvk, dtype=np.float32)
    b = np.asarray(b_qvk, dtype=np.float32)
    xt = x.T.astype(ml_dtypes.bfloat16)  # (D, N)

    def pack(a):  # (D, M) -> (128, KT*M), partition-contiguous
        kt, m = a.shape[0] // 128, a.shape[1]
        return np.ascontiguousarray(
            a.reshape(kt, 128, m).transpose(1, 0, 2).reshape(128, kt * m)
        )

    xtp = [pack(xt[:, tt * TT : (tt + 1) * TT]) for tt in range(NTT)]
    xt1 = xtp[1]
    xt23 = np.ascontiguousarray(np.concatenate([xtp[2], xtp[3]], axis=1))
    ident = np.eye(128, dtype=ml_dtypes.bfloat16)

    tri = np.tril(np.ones((C, C), dtype=np.float32))  # [i, j] valid j<=i
    mask = np.ascontiguousarray(tri.T.astype(ml_dtypes.bfloat16))  # [j, i]

    in_maps = []
    for core in range(NCORES):
        heads = [HPC * core + i for i in range(HPC)]
        # torch.chunk order in reference: q, v, k
        qcols = np.concatenate([np.arange(E * h, E * h + E) for h in heads])
        vcols = qcols + D
        kcols = qcols + 2 * D
        Wc = np.concatenate(
            [W[:, qcols], W[:, kcols], W[:, vcols]], axis=1
        ).astype(ml_dtypes.bfloat16)  # (D, 384)
        # mega: [ident | (Wq|Wk|Wv | xt0_k) x 8]
        parts = [ident]
        for k in range(KT):
            parts.append(Wc[k * 128 : (k + 1) * 128, :])
            parts.append(xtp[0][:, k * TT : (k + 1) * TT])
        mega = np.ascontiguousarray(np.concatenate(parts, axis=1))
        bq = b[qcols].reshape(128, 1)
        bk = b[kcols].reshape(128, 1)
        bv = b[vcols].reshape(128, 1)
        cf = np.ascontiguousarray(
            np.concatenate([bq, bk, bv], axis=1, dtype=np.float32)
        )
        in_maps.append(dict(mega=mega, cf=cf, msk=mask, xt1=xt1, xt23=xt23))
    return in_maps


_CACHE = {}


def kernel(x, W_qvk, b_qvk, head_num):
    assert int(np.asarray(head_num)) == H
    if "nc" not in _CACHE:
        _CACHE["nc"] = build_nc()
    nc = _CACHE["nc"]
    in_maps = host_inputs(x, W_qvk, b_qvk)
    from concourse.bass_utils import run_bass_kernel_spmd

    res = run_bass_kernel_spmd(
        nc,
        in_maps,
        core_ids=list(range(NCORES)),
        trace=bool(int(os.environ.get("KERNEL_TRACE", "0"))),
    )
    _CACHE["last_result"] = res
    out = np.concatenate([r["out"] for r in res.results], axis=1)
    return out.reshape(B, N, D).astype(np.float32)
